# revision 2
# baseline (speedup 1.0000x reference)
"""LPO loss kernel v2 for 8 TRN2 NeuronCores.

Math (B=256, D=64, S=32):
  zs[j,d,s] = post_mean[j,d] + eps[j,d,s]*exp(0.5*post_logvar[j,d])
  logp_post[i,j,d,s] = A0[i,d] + A1[i,d]*z + A2[i,d]*z^2
  lagg[j,d,s] = log(sum_i exp(logp_post)) - log(B)
  kl = sum_{j,d,s}(lagg - logp_prior) / (B*S)

Sharding: j split 8 ways; i-reduction local; host combines scalars.

v2 dataflow (per core), swapped-operand layout:
  TensorE mains: out[i(128 of 256), js 512] = amat[6, i128]^T @ zmat[6, js512]
                 (bf16 hi/lo of A1*z + A2*z^2; A0 folded into exp bias)
  exp:   ScalarE activation(Exp, bias=A0col) on ~62.5% of (d, ihalf) tiles
         DVE Schraudolph on the rest: u32((ps + A0 + B/A) * A) bitcast f32r
  i-fold: TensorE replicated-ones matmul [32,512] blocks, 2 accumulating
          matmuls (i-halves) per js chunk; 4 pairs fill a PSUM bank, then
          one Ln+accum per bank (host divides the 32x replication out)
  A-coefficients (A0/A1/A2 hi-lo splits) are precomputed on host ([B,D]
  sized, negligible); z/z2 and the prior logpdf run on device.
"""

import sys

sys.path.insert(0, "/opt/trn_rl_repo")

import numpy as np
import ml_dtypes

import concourse.bass as bass
import concourse.bacc as bacc
import concourse.mybir as mybir
from concourse import tile
from concourse.bass_utils import run_bass_kernel_spmd

# Route both Exp and Ln to the one table set that holds them both
# ("natural_log_exp_and_others") so the kernel pays a single
# ACT_TABLE_LOAD instead of swapping sets between exp and ln calls.
# Order of the dict is preserved, so act_func_set_ids stay valid.
_orig_get_tables = bacc.get_activation_tables


def _patched_get_tables(arch):
    tabs = dict(_orig_get_tables(arch))
    exp_ln = {mybir.ActivationFunctionType.Exp, mybir.ActivationFunctionType.Ln}
    combined = None
    for name, fns in tabs.items():
        if exp_ln <= fns:
            combined = name
            break
    if combined is not None:
        for name in tabs:
            if name != combined:
                tabs[name] = tabs[name] - exp_ln
    return tabs


bacc.get_activation_tables = _patched_get_tables

B, D, S = 256, 64, 32
NCORES = 8
BJ = B // NCORES          # 32 j's per core
JS = BJ * S               # 1024 js columns per core
J2 = JS // 2              # [D, JS] reshaped to [128, J2] (same bytes)
DQ = D // 4               # 16 d's per zmat quarter
L2P = float(np.log(2.0 * np.pi))
LN2 = float(np.log(2.0))
C0 = -0.5 * L2P
F32 = mybir.dt.float32
F32R = mybir.dt.float32r
BF16 = mybir.dt.bfloat16
U16 = mybir.dt.uint16
AF = mybir.ActivationFunctionType
ALU = mybir.AluOpType
bf = ml_dtypes.bfloat16

# Schraudolph exp in bf16 bit-space: exp(x) ~= bitcast_bf16(u16(x*A + B));
# the fp32->uint convert saturates negatives to 0 on TRN2 (verified on hw),
# which handles exp underflow exactly.
A_SCH = float(2.0**7 / np.log(2.0))
C_SCH = 7.37              # mean-error-nulling constant (tuned numerically)
B_SCH = float(127 * 2**7) - C_SCH


def _dve_tile(d, ih):
    # which (d, i-half) exp tiles go to DVE-Schraudolph: 37.5% of tiles
    return ih == 1 and (d % 4 != 0)


_CACHED_NC = None


def _build_nc(reps=1):
    """reps>1 repeats the whole compute (idempotent) inside one NEFF so
    wall-clock deltas between rep counts isolate true HW exec time."""
    nc = bacc.Bacc(None)

    for val in (0.5, LN2):
        t = nc.alloc_sbuf_tensor(f"const-user-{val}", [128, 1], F32)
        nc.gpsimd.memset(t.ap(), val)
        nc.const_aps.aps[(F32, val)] = t.ap()
    nc.all_engine_barrier()

    eps64 = nc.declare_dram_parameter("eps64", [128, J2], F32, isOutput=False)
    mj64 = nc.declare_dram_parameter("mj64", [128, J2], F32, isOutput=False)
    lvj64 = nc.declare_dram_parameter("lvj64", [128, J2], F32, isOutput=False)
    pm64 = nc.declare_dram_parameter("pm64", [128, J2], F32, isOutput=False)
    plv64 = nc.declare_dram_parameter("plv64", [128, J2], F32, isOutput=False)
    amat_p = nc.declare_dram_parameter("amat_p", [6, D * B], BF16, isOutput=False)
    a0c_p = nc.declare_dram_parameter("a0c_p", [128, D * 2], F32, isOutput=False)
    abc_p = nc.declare_dram_parameter("abc_p", [128, D * 2], F32, isOutput=False)
    out = nc.declare_dram_parameter("out", [128, 2], F32, isOutput=True)

    with tile.TileContext(nc) as tc:
        with (
            tc.tile_pool(name="persist", bufs=1) as pp,
            tc.tile_pool(name="prep", bufs=1) as prep,
            tc.tile_pool(name="dram", bufs=1, space="DRAM") as dram,
            tc.tile_pool(name="psum", bufs=3, space="PSUM") as psp,
            tc.tile_pool(name="snk", bufs=2, space="PSUM") as snk,
            tc.tile_pool(name="expp", bufs=4) as expp,
            tc.tile_pool(name="sup", bufs=3) as sup,
            tc.tile_pool(name="lasc", bufs=2) as lasc,
            tc.tile_pool(name="zqp", bufs=2) as zqp,
        ):
            # ---------------- persistent tensors ----------------
            amat = pp.tile([6, D * B], BF16, tag="amat")        # 32KB rows 0-5
            a0c = pp.tile([128, D * 2], F32, tag="a0c")         # A0[i,(d,ih)]
            abc = pp.tile([128, D * 2], F32, tag="abc")         # A0 + B/A
            acst = pp.tile([128, JS], F32, tag="acst")          # A_SCH const
            ones_b = pp.tile([128, 32], BF16, tag="ones_b")
            acc_cols = pp.tile([128, 32], F32, tag="acc_cols")
            outt = pp.tile([128, 2], F32, tag="outt")

            nc.gpsimd.memset(ones_b[:, :], 1.0)
            nc.gpsimd.memset(acst[:, :], A_SCH)

            # DRAM bounce buffers for the partition->free zmat row gathers
            # (the "(o p) j -> o (p j)" rearrange needs linear memory)
            zh_d = dram.tile([128, J2], BF16, tag="zh_d")
            zl_d = dram.tile([128, J2], BF16, tag="zl_d")
            z2h_d = dram.tile([128, J2], BF16, tag="z2h_d")
            z2l_d = dram.tile([128, J2], BF16, tag="z2l_d")

            # ---------------- prep tiles ----------------
            t_eps = prep.tile([128, J2], F32, tag="t_eps")
            t_mj = prep.tile([128, J2], F32, tag="t_mj")
            t_lvj = prep.tile([128, J2], F32, tag="t_lvj")
            t_pm = prep.tile([128, J2], F32, tag="t_pm")
            t_plv = prep.tile([128, J2], F32, tag="t_plv")
            t_std = prep.tile([128, J2], F32, tag="t_std")
            t_z = prep.tile([128, J2], F32, tag="t_z")
            t_z2 = prep.tile([128, J2], F32, tag="t_z2")
            t_r = prep.tile([128, J2], F32, tag="t_r")
            t_lp = prep.tile([128, J2], F32, tag="t_lp")
            t_wp = prep.tile([128, J2], F32, tag="t_wp")
            zh = prep.tile([128, J2], BF16, tag="zh")
            zl = prep.tile([128, J2], BF16, tag="zl")
            z2h = prep.tile([128, J2], BF16, tag="z2h")
            z2l = prep.tile([128, J2], BF16, tag="z2l")
            lpsum = prep.tile([128, 1], F32, tag="lpsum")

            # loads needed before the main loop
            nc.sync.dma_start(t_eps[:, :], eps64[:, :])
            nc.sync.dma_start(t_mj[:, :], mj64[:, :])
            nc.sync.dma_start(t_lvj[:, :], lvj64[:, :])
            nc.sync.dma_start(amat[:, :], amat_p[:, :])
            nc.sync.dma_start(a0c[:, :], a0c_p[:, :])
            nc.sync.dma_start(abc[:, :], abc_p[:, :])

            # ---------------- z path ----------------
            # z = mj + eps*exp(0.5*lvj); z2 = z*z
            nc.scalar.activation(t_std[:, :], t_lvj[:, :], AF.Exp, scale=0.5)
            nc.vector.tensor_mul(t_z[:, :], t_eps[:, :], t_std[:, :])
            nc.vector.tensor_add(t_z[:, :], t_z[:, :], t_mj[:, :])
            nc.vector.tensor_mul(t_z2[:, :], t_z[:, :], t_z[:, :])
            # hi/lo splits
            nc.vector.tensor_copy(zh[:, :], t_z[:, :])
            nc.vector.tensor_sub(t_r[:, :], t_z[:, :], zh[:, :])
            nc.vector.tensor_copy(zl[:, :], t_r[:, :])
            nc.vector.tensor_copy(z2h[:, :], t_z2[:, :])
            nc.vector.tensor_sub(t_r[:, :], t_z2[:, :], z2h[:, :])
            nc.vector.tensor_copy(z2l[:, :], t_r[:, :])
            nc.sync.dma_start(zh_d[:, :], zh[:, :])
            nc.sync.dma_start(zl_d[:, :], zl[:, :])
            nc.sync.dma_start(z2h_d[:, :], z2h[:, :])
            nc.sync.dma_start(z2l_d[:, :], z2l[:, :])

            # ---------------- main loop ----------------
            def build_zmat_quarter(qi):
                # rows 0,1 = zh ; 2 = zl ; 3,4 = z2h ; 5 = z2l
                # (quarter qi = DRAM partitions [qi*32, qi*32+32))
                zq = zqp.tile([6, DQ * JS], BF16, tag="zq", name="zq")
                psl = slice(qi * 32, (qi + 1) * 32)
                for row, src in ((0, zh_d), (1, zh_d), (2, zl_d),
                                 (3, z2h_d), (4, z2h_d), (5, z2l_d)):
                    nc.sync.dma_start(
                        zq[row:row + 1, :],
                        src[psl, :].rearrange("(o p) j -> o (p j)", o=1))
                return zq

            sums_bank = [None]  # current [128, 512] PSUM sums bank

            def issue_ones(d, rhs_slc):
                # i-fold: 2 accumulating ones-matmuls per js chunk;
                # pair q=(d*2+jc) lands in 32-part block q%4 of the bank
                for jc in range(2):
                    q = d * 2 + jc
                    blk = q % 4
                    if blk == 0:
                        sums_bank[0] = snk.tile([128, 512], F32,
                                                tag="snk", name="snk")
                    for ih in range(2):
                        nc.tensor.matmul(
                            sums_bank[0][blk * 32:(blk + 1) * 32, :],
                            ones_b[:, :],
                            rhs_slc[ih][:, jc * 512:(jc + 1) * 512],
                            start=(ih == 0), stop=(ih == 1),
                            tile_position=(0, blk * 32))
                if d % 2 == 1:
                    # bank (d-1)//2 is complete: Ln + accum -> acc column
                    b = (d - 1) // 2
                    lsc = lasc.tile([128, 512], BF16, tag="lsc", name="lsc")
                    nc.scalar.activation(lsc[:, :], sums_bank[0][:, :], AF.Ln,
                                         accum_out=acc_cols[:, b:b + 1])
                    sums_bank[0] = None

            pend = None       # (d, rhs_slc) whose ones-fold is deferred
            zq = None
            for d in range(D):
                dq = d % DQ
                if dq == 0:
                    zq = build_zmat_quarter(d // DQ)
                # per (d, i-half): matmul both js chunks, then exp
                rhs_slc = [None, None]   # rhs APs for the ones-fold
                for ih in range(2):
                    ps = psp.tile([128, 2 * 512], F32, tag="ps")
                    for jc in range(2):
                        nc.tensor.matmul(
                            ps[:, jc * 512:(jc + 1) * 512],
                            amat[0:6, d * B + ih * 128:
                                 d * B + ih * 128 + 128],
                            zq[0:6, dq * JS + jc * 512:
                               dq * JS + (jc + 1) * 512],
                            start=True, stop=True)
                    col = d * 2 + ih
                    if _dve_tile(d, ih):
                        su = sup.tile([128, 2 * 512], U16, tag="su")
                        nc.vector.scalar_tensor_tensor(
                            su[:, :], ps[:, :], abc[:, col:col + 1],
                            acst[:, 0:1024],
                            op0=ALU.add, op1=ALU.mult)
                        rhs_slc[ih] = su[:, :].bitcast(BF16)
                    else:
                        ex = expp.tile([128, 2 * 512], BF16, tag="ex")
                        nc.scalar.activation(ex[:, :], ps[:, :], AF.Exp,
                                             bias=a0c[:, col:col + 1])
                        rhs_slc[ih] = ex[:, :]
                # software pipelining: fold the PREVIOUS d now, so the PE
                # never waits on this d's exp in program order
                if pend is not None:
                    issue_ones(*pend)
                pend = (d, rhs_slc)
            issue_ones(*pend)

            # ---- prior logpdf + partial sum (overlaps the loop tail)
            # lp = C0 -0.5*plv -(z-pm)^2/(2e^plv+1e-4)
            nc.sync.dma_start(t_pm[:, :], pm64[:, :])
            nc.sync.dma_start(t_plv[:, :], plv64[:, :])
            nc.scalar.activation(t_wp[:, :], t_plv[:, :], AF.Exp, bias=LN2)
            nc.vector.tensor_scalar_add(t_wp[:, :], t_wp[:, :], 1e-4)
            nc.vector.reciprocal(t_wp[:, :], t_wp[:, :])
            nc.vector.tensor_sub(t_lp[:, :], t_z[:, :], t_pm[:, :])
            nc.vector.tensor_mul(t_lp[:, :], t_lp[:, :], t_lp[:, :])
            nc.vector.tensor_mul(t_lp[:, :], t_lp[:, :], t_wp[:, :])
            nc.vector.tensor_scalar(t_plv[:, :], t_plv[:, :], -0.5, C0,
                                    op0=ALU.mult, op1=ALU.add)
            nc.vector.tensor_sub(t_lp[:, :], t_plv[:, :], t_lp[:, :])
            nc.vector.reduce_sum(lpsum[:, :], t_lp[:, :],
                                 axis=mybir.AxisListType.X)

            # ---------------- tail: combine partials ----------------
            nc.vector.reduce_sum(outt[:, 0:1], acc_cols[:, :],
                                 axis=mybir.AxisListType.X)
            nc.vector.tensor_copy(outt[:, 1:2], lpsum[:, :])
            nc.sync.dma_start(out[:, :], outt[:, :])

    nc.compile()
    return nc


def _host_coeffs(post_mean, post_logvar):
    """amat rows (A1h, A1l, A1h, A2h, A2l, A2h) in [6, (d, i)] bf16 layout,
    plus A0 bias columns a0c[p, (d, ih)] and abc = a0c + B/A."""
    m = post_mean.astype(np.float64)        # [B, D]
    lv = post_logvar.astype(np.float64)
    w = 1.0 / (2.0 * np.exp(lv) + 1e-4)
    A1 = 2.0 * m * w
    A2 = -w
    A0 = C0 - 0.5 * lv - m * m * w

    def hilo(x):  # [B, D] -> bf16 hi + lo
        hi = x.astype(np.float32).astype(bf)
        lo = (x - hi.astype(np.float64)).astype(np.float32).astype(bf)
        return hi, lo

    a1h, a1l = hilo(A1)
    a2h, a2l = hilo(A2)
    # amat row layout: free index = d*B + i  (i = ih*128 + p)
    rows = [a1h, a1l, a1h, a2h, a2l, a2h]
    amat = np.stack([np.ascontiguousarray(r.T).reshape(-1) for r in rows])
    # a0c[p, d*2 + ih] = A0[ih*128 + p, d]
    a0c = np.empty((128, D * 2), np.float32)
    for ih in range(2):
        a0c[:, ih::2] = A0[ih * 128:(ih + 1) * 128, :].astype(np.float32)
    abc = (a0c.astype(np.float64) + B_SCH / A_SCH).astype(np.float32)
    return amat.astype(bf), a0c, abc


def _prep_core_inputs(prior_mean, prior_logvar, post_mean, post_logvar, eps,
                      c, coeffs):
    jsl = slice(c * BJ, (c + 1) * BJ)

    def b64(x):  # [BJ, D] -> [D, JS] broadcast over s -> [128, J2]
        return np.ascontiguousarray(
            np.broadcast_to(x.T[:, :, None], (D, BJ, S)).reshape(128, J2),
            dtype=np.float32)

    amat, a0c, abc = coeffs
    eps64 = np.ascontiguousarray(
        eps[jsl].transpose(1, 0, 2).reshape(128, J2), dtype=np.float32)
    return {
        "eps64": eps64,
        "mj64": b64(post_mean[jsl]),
        "lvj64": b64(post_logvar[jsl]),
        "pm64": b64(prior_mean[jsl]),
        "plv64": b64(prior_logvar[jsl]),
        "amat_p": amat,
        "a0c_p": a0c,
        "abc_p": abc,
    }


_RUN_KWARGS = {}
_LAST_RESULT = None


def kernel(prior_mean, prior_logvar, post_mean, post_logvar, eps):
    global _CACHED_NC, _LAST_RESULT
    prior_mean = np.asarray(prior_mean, dtype=np.float32)
    prior_logvar = np.asarray(prior_logvar, dtype=np.float32)
    post_mean = np.asarray(post_mean, dtype=np.float32)
    post_logvar = np.asarray(post_logvar, dtype=np.float32)
    eps = np.asarray(eps, dtype=np.float32)

    if _CACHED_NC is None:
        _CACHED_NC = _build_nc()
    nc = _CACHED_NC

    coeffs = _host_coeffs(post_mean, post_logvar)
    in_maps = [
        _prep_core_inputs(prior_mean, prior_logvar, post_mean, post_logvar,
                          eps, c, coeffs)
        for c in range(NCORES)
    ]
    res = run_bass_kernel_spmd(nc, in_maps, core_ids=list(range(NCORES)),
                               **_RUN_KWARGS)
    _LAST_RESULT = res

    tot = 0.0
    for c in range(NCORES):
        o = np.asarray(res.results[c]["out"], dtype=np.float64)
        # log-sum column is 32x replicated across each partition block
        tot += o[:, 0].sum() / 32.0 - o[:, 1].sum()
    kl = (tot - B * D * S * np.log(B)) / (B * S)
    return np.float32(kl)


# revision 3
# speedup vs baseline: 1.8241x; 1.8241x over previous
"""LPO loss kernel v2 for 8 TRN2 NeuronCores.

Math (B=256, D=64, S=32):
  zs[j,d,s] = post_mean[j,d] + eps[j,d,s]*exp(0.5*post_logvar[j,d])
  logp_post[i,j,d,s] = A0[i,d] + A1[i,d]*z + A2[i,d]*z^2
  lagg[j,d,s] = log(sum_i exp(logp_post)) - log(B)
  kl = sum_{j,d,s}(lagg - logp_prior) / (B*S)

Sharding: j split 8 ways; i-reduction local; host combines scalars.

v2 dataflow (per core), swapped-operand layout:
  TensorE mains: out[i(128 of 256), js 512] = amat[6, i128]^T @ zmat[6, js512]
                 (bf16 hi/lo of A1*z + A2*z^2; A0 folded into exp bias)
  exp:   ScalarE activation(Exp, bias=A0col) on ~62.5% of (d, ihalf) tiles
         DVE Schraudolph on the rest, in bf16 bit-space:
         u16((ps + A0 + B/A) * 2^7/ln2) bitcast to bf16 (fp32->uint
         converts saturate negatives to 0 on TRN2 = exact underflow)
  i-fold: TensorE replicated-ones matmul [32,512] blocks, 2 accumulating
          matmuls (i-halves) per js chunk; 4 pairs fill a PSUM bank, then
          one Ln+accum per bank (host divides the 32x replication out)
  A-coefficients (A0/A1/A2 hi-lo splits) are precomputed on host ([B,D]
  sized, negligible); z/z2 and the prior logpdf run on device.
"""

import sys

sys.path.insert(0, "/opt/trn_rl_repo")

import numpy as np
import ml_dtypes

import concourse.bass as bass
import concourse.bacc as bacc
import concourse.mybir as mybir
from concourse import tile
from concourse.bass_utils import run_bass_kernel_spmd

# Route both Exp and Ln to the one table set that holds them both
# ("natural_log_exp_and_others") so the kernel pays a single
# ACT_TABLE_LOAD instead of swapping sets between exp and ln calls.
# Order of the dict is preserved, so act_func_set_ids stay valid.
_orig_get_tables = bacc.get_activation_tables


def _patched_get_tables(arch):
    tabs = dict(_orig_get_tables(arch))
    exp_ln = {mybir.ActivationFunctionType.Exp, mybir.ActivationFunctionType.Ln}
    combined = None
    for name, fns in tabs.items():
        if exp_ln <= fns:
            combined = name
            break
    if combined is not None:
        for name in tabs:
            if name != combined:
                tabs[name] = tabs[name] - exp_ln
    return tabs


bacc.get_activation_tables = _patched_get_tables

B, D, S = 256, 64, 32
NCORES = 8
BJ = B // NCORES          # 32 j's per core
JS = BJ * S               # 1024 js columns per core
J2 = JS // 2              # [D, JS] reshaped to [128, J2] (same bytes)
DQ = D // 4               # 16 d's per zmat quarter
L2P = float(np.log(2.0 * np.pi))
LN2 = float(np.log(2.0))
C0 = -0.5 * L2P
F32 = mybir.dt.float32
F32R = mybir.dt.float32r
BF16 = mybir.dt.bfloat16
U16 = mybir.dt.uint16
AF = mybir.ActivationFunctionType
ALU = mybir.AluOpType
bf = ml_dtypes.bfloat16

# Schraudolph exp in bf16 bit-space: exp(x) ~= bitcast_bf16(u16(x*A + B));
# the fp32->uint convert saturates negatives to 0 on TRN2 (verified on hw),
# which handles exp underflow exactly.
A_SCH = float(2.0**7 / np.log(2.0))
C_SCH = 7.37              # mean-error-nulling constant (tuned numerically)
B_SCH = float(127 * 2**7) - C_SCH


def _dve_tile(d, ih):
    # which (d, i-half) exp tiles go to DVE-Schraudolph: 37.5% of tiles
    return ih == 1 and (d % 4 != 0)


_CACHED_NC = None


def _build_nc(reps=1):
    """reps>1 repeats the whole compute (idempotent) inside one NEFF so
    wall-clock deltas between rep counts isolate true HW exec time."""
    nc = bacc.Bacc(None)

    for val in (0.5, LN2):
        t = nc.alloc_sbuf_tensor(f"const-user-{val}", [128, 1], F32)
        nc.gpsimd.memset(t.ap(), val)
        nc.const_aps.aps[(F32, val)] = t.ap()
    nc.all_engine_barrier()

    eps64 = nc.declare_dram_parameter("eps64", [128, J2], F32, isOutput=False)
    mj64 = nc.declare_dram_parameter("mj64", [128, J2], F32, isOutput=False)
    lvj64 = nc.declare_dram_parameter("lvj64", [128, J2], F32, isOutput=False)
    pm64 = nc.declare_dram_parameter("pm64", [128, J2], F32, isOutput=False)
    plv64 = nc.declare_dram_parameter("plv64", [128, J2], F32, isOutput=False)
    amat_p = nc.declare_dram_parameter("amat_p", [6, D * B], BF16, isOutput=False)
    a0c_p = nc.declare_dram_parameter("a0c_p", [128, D * 2], F32, isOutput=False)
    abc_p = nc.declare_dram_parameter("abc_p", [128, D * 2], F32, isOutput=False)
    out = nc.declare_dram_parameter("out", [128, 2], F32, isOutput=True)

    with tile.TileContext(nc) as tc:
        with (
            tc.tile_pool(name="persist", bufs=1) as pp,
            tc.tile_pool(name="prep", bufs=1) as prep,
            tc.tile_pool(name="dram", bufs=1, space="DRAM") as dram,
            tc.tile_pool(name="psum", bufs=3, space="PSUM") as psp,
            tc.tile_pool(name="snk", bufs=2, space="PSUM") as snk,
            tc.tile_pool(name="expp", bufs=4) as expp,
            tc.tile_pool(name="sup", bufs=3) as sup,
            tc.tile_pool(name="lasc", bufs=2) as lasc,
            tc.tile_pool(name="zqp", bufs=2) as zqp,
        ):
            # ---------------- persistent tensors ----------------
            amat = pp.tile([6, D * B], BF16, tag="amat")        # 32KB rows 0-5
            a0c = pp.tile([128, D * 2], F32, tag="a0c")         # A0[i,(d,ih)]
            abc = pp.tile([128, D * 2], F32, tag="abc")         # A0 + B/A
            acst = pp.tile([128, JS], F32, tag="acst")          # A_SCH const
            ones_b = pp.tile([128, 32], BF16, tag="ones_b")
            acc_cols = pp.tile([128, 32], F32, tag="acc_cols")
            outt = pp.tile([128, 2], F32, tag="outt")

            nc.gpsimd.memset(ones_b[:, :], 1.0)
            nc.gpsimd.memset(acst[:, :], A_SCH)

            # DRAM bounce buffers for the partition->free zmat row gathers
            # (the "(o p) j -> o (p j)" rearrange needs linear memory)
            zh_d = dram.tile([128, J2], BF16, tag="zh_d")
            zl_d = dram.tile([128, J2], BF16, tag="zl_d")
            z2h_d = dram.tile([128, J2], BF16, tag="z2h_d")
            z2l_d = dram.tile([128, J2], BF16, tag="z2l_d")

            # ---------------- prep tiles ----------------
            t_eps = prep.tile([128, J2], F32, tag="t_eps")
            t_mj = prep.tile([128, J2], F32, tag="t_mj")
            t_lvj = prep.tile([128, J2], F32, tag="t_lvj")
            t_pm = prep.tile([128, J2], F32, tag="t_pm")
            t_plv = prep.tile([128, J2], F32, tag="t_plv")
            t_std = prep.tile([128, J2], F32, tag="t_std")
            t_z = prep.tile([128, J2], F32, tag="t_z")
            t_z2 = prep.tile([128, J2], F32, tag="t_z2")
            t_r = prep.tile([128, J2], F32, tag="t_r")
            t_lp = prep.tile([128, J2], F32, tag="t_lp")
            t_wp = prep.tile([128, J2], F32, tag="t_wp")
            zh = prep.tile([128, J2], BF16, tag="zh")
            zl = prep.tile([128, J2], BF16, tag="zl")
            z2h = prep.tile([128, J2], BF16, tag="z2h")
            z2l = prep.tile([128, J2], BF16, tag="z2l")
            lpsum = prep.tile([128, 1], F32, tag="lpsum")

            # loads needed before the main loop
            nc.sync.dma_start(t_eps[:, :], eps64[:, :])
            nc.sync.dma_start(t_mj[:, :], mj64[:, :])
            nc.sync.dma_start(t_lvj[:, :], lvj64[:, :])
            nc.sync.dma_start(amat[:, :], amat_p[:, :])
            nc.sync.dma_start(a0c[:, :], a0c_p[:, :])
            nc.sync.dma_start(abc[:, :], abc_p[:, :])

            # ---------------- z path ----------------
            # z = mj + eps*exp(0.5*lvj); z2 = z*z
            nc.scalar.activation(t_std[:, :], t_lvj[:, :], AF.Exp, scale=0.5)
            nc.vector.tensor_mul(t_z[:, :], t_eps[:, :], t_std[:, :])
            nc.vector.tensor_add(t_z[:, :], t_z[:, :], t_mj[:, :])
            nc.vector.tensor_mul(t_z2[:, :], t_z[:, :], t_z[:, :])
            # hi/lo splits
            nc.vector.tensor_copy(zh[:, :], t_z[:, :])
            nc.vector.tensor_sub(t_r[:, :], t_z[:, :], zh[:, :])
            nc.vector.tensor_copy(zl[:, :], t_r[:, :])
            nc.vector.tensor_copy(z2h[:, :], t_z2[:, :])
            nc.vector.tensor_sub(t_r[:, :], t_z2[:, :], z2h[:, :])
            nc.vector.tensor_copy(z2l[:, :], t_r[:, :])
            nc.sync.dma_start(zh_d[:, :], zh[:, :])
            nc.sync.dma_start(zl_d[:, :], zl[:, :])
            nc.sync.dma_start(z2h_d[:, :], z2h[:, :])
            nc.sync.dma_start(z2l_d[:, :], z2l[:, :])

            # ---------------- main loop ----------------
            def build_zmat_quarter(qi):
                # rows 0,1 = zh ; 2 = zl ; 3,4 = z2h ; 5 = z2l
                # (quarter qi = DRAM partitions [qi*32, qi*32+32))
                zq = zqp.tile([6, DQ * JS], BF16, tag="zq", name="zq")
                psl = slice(qi * 32, (qi + 1) * 32)
                for row, src in ((0, zh_d), (1, zh_d), (2, zl_d),
                                 (3, z2h_d), (4, z2h_d), (5, z2l_d)):
                    nc.sync.dma_start(
                        zq[row:row + 1, :],
                        src[psl, :].rearrange("(o p) j -> o (p j)", o=1))
                return zq

            sums_bank = [None]  # current [128, 512] PSUM sums bank

            def issue_ones(d, rhs_slc):
                # i-fold: 2 accumulating ones-matmuls per js chunk;
                # pair q=(d*2+jc) lands in 32-part block q%4 of the bank
                for jc in range(2):
                    q = d * 2 + jc
                    blk = q % 4
                    if blk == 0:
                        sums_bank[0] = snk.tile([128, 512], F32,
                                                tag="snk", name="snk")
                    for ih in range(2):
                        nc.tensor.matmul(
                            sums_bank[0][blk * 32:(blk + 1) * 32, :],
                            ones_b[:, :],
                            rhs_slc[ih][:, jc * 512:(jc + 1) * 512],
                            start=(ih == 0), stop=(ih == 1),
                            tile_position=(0, blk * 32))
                if d % 2 == 1:
                    # bank (d-1)//2 is complete: Ln + accum -> acc column
                    b = (d - 1) // 2
                    lsc = lasc.tile([128, 512], BF16, tag="lsc", name="lsc")
                    nc.scalar.activation(lsc[:, :], sums_bank[0][:, :], AF.Ln,
                                         accum_out=acc_cols[:, b:b + 1])
                    sums_bank[0] = None

            pend = None       # (d, rhs_slc) whose ones-fold is deferred
            zq = None
            for d in range(D):
                dq = d % DQ
                if dq == 0:
                    zq = build_zmat_quarter(d // DQ)
                # per (d, i-half): matmul both js chunks, then exp
                rhs_slc = [None, None]   # rhs APs for the ones-fold
                for ih in range(2):
                    ps = psp.tile([128, 2 * 512], F32, tag="ps")
                    for jc in range(2):
                        nc.tensor.matmul(
                            ps[:, jc * 512:(jc + 1) * 512],
                            amat[0:6, d * B + ih * 128:
                                 d * B + ih * 128 + 128],
                            zq[0:6, dq * JS + jc * 512:
                               dq * JS + (jc + 1) * 512],
                            start=True, stop=True)
                    col = d * 2 + ih
                    if _dve_tile(d, ih):
                        su = sup.tile([128, 2 * 512], U16, tag="su")
                        nc.vector.scalar_tensor_tensor(
                            su[:, :], ps[:, :], abc[:, col:col + 1],
                            acst[:, 0:1024],
                            op0=ALU.add, op1=ALU.mult)
                        rhs_slc[ih] = su[:, :].bitcast(BF16)
                    else:
                        ex = expp.tile([128, 2 * 512], BF16, tag="ex")
                        nc.scalar.activation(ex[:, :], ps[:, :], AF.Exp,
                                             bias=a0c[:, col:col + 1])
                        rhs_slc[ih] = ex[:, :]
                # software pipelining: fold the PREVIOUS d now, so the PE
                # never waits on this d's exp in program order
                if pend is not None:
                    issue_ones(*pend)
                pend = (d, rhs_slc)
            issue_ones(*pend)

            # ---- prior logpdf + partial sum (overlaps the loop tail)
            # lp = C0 -0.5*plv -(z-pm)^2/(2e^plv+1e-4)
            nc.sync.dma_start(t_pm[:, :], pm64[:, :])
            nc.sync.dma_start(t_plv[:, :], plv64[:, :])
            nc.scalar.activation(t_wp[:, :], t_plv[:, :], AF.Exp, bias=LN2)
            nc.vector.tensor_scalar_add(t_wp[:, :], t_wp[:, :], 1e-4)
            nc.vector.reciprocal(t_wp[:, :], t_wp[:, :])
            nc.vector.tensor_sub(t_lp[:, :], t_z[:, :], t_pm[:, :])
            nc.vector.tensor_mul(t_lp[:, :], t_lp[:, :], t_lp[:, :])
            nc.vector.tensor_mul(t_lp[:, :], t_lp[:, :], t_wp[:, :])
            nc.vector.tensor_scalar(t_plv[:, :], t_plv[:, :], -0.5, C0,
                                    op0=ALU.mult, op1=ALU.add)
            nc.vector.tensor_sub(t_lp[:, :], t_plv[:, :], t_lp[:, :])
            nc.vector.reduce_sum(lpsum[:, :], t_lp[:, :],
                                 axis=mybir.AxisListType.X)

            # ---------------- tail: combine partials ----------------
            nc.vector.reduce_sum(outt[:, 0:1], acc_cols[:, :],
                                 axis=mybir.AxisListType.X)
            nc.vector.tensor_copy(outt[:, 1:2], lpsum[:, :])
            nc.sync.dma_start(out[:, :], outt[:, :])

    nc.compile()
    return nc


def _host_coeffs(post_mean, post_logvar):
    """amat rows (A1h, A1l, A1h, A2h, A2l, A2h) in [6, (d, i)] bf16 layout,
    plus A0 bias columns a0c[p, (d, ih)] and abc = a0c + B/A."""
    m = post_mean.astype(np.float64)        # [B, D]
    lv = post_logvar.astype(np.float64)
    w = 1.0 / (2.0 * np.exp(lv) + 1e-4)
    A1 = 2.0 * m * w
    A2 = -w
    A0 = C0 - 0.5 * lv - m * m * w

    def hilo(x):  # [B, D] -> bf16 hi + lo
        hi = x.astype(np.float32).astype(bf)
        lo = (x - hi.astype(np.float64)).astype(np.float32).astype(bf)
        return hi, lo

    a1h, a1l = hilo(A1)
    a2h, a2l = hilo(A2)
    # amat row layout: free index = d*B + i  (i = ih*128 + p)
    rows = [a1h, a1l, a1h, a2h, a2l, a2h]
    amat = np.stack([np.ascontiguousarray(r.T).reshape(-1) for r in rows])
    # a0c[p, d*2 + ih] = A0[ih*128 + p, d]
    a0c = np.empty((128, D * 2), np.float32)
    for ih in range(2):
        a0c[:, ih::2] = A0[ih * 128:(ih + 1) * 128, :].astype(np.float32)
    abc = (a0c.astype(np.float64) + B_SCH / A_SCH).astype(np.float32)
    return amat.astype(bf), a0c, abc


def _prep_core_inputs(prior_mean, prior_logvar, post_mean, post_logvar, eps,
                      c, coeffs):
    jsl = slice(c * BJ, (c + 1) * BJ)

    def b64(x):  # [BJ, D] -> [D, JS] broadcast over s -> [128, J2]
        return np.ascontiguousarray(
            np.broadcast_to(x.T[:, :, None], (D, BJ, S)).reshape(128, J2),
            dtype=np.float32)

    amat, a0c, abc = coeffs
    eps64 = np.ascontiguousarray(
        eps[jsl].transpose(1, 0, 2).reshape(128, J2), dtype=np.float32)
    return {
        "eps64": eps64,
        "mj64": b64(post_mean[jsl]),
        "lvj64": b64(post_logvar[jsl]),
        "pm64": b64(prior_mean[jsl]),
        "plv64": b64(prior_logvar[jsl]),
        "amat_p": amat,
        "a0c_p": a0c,
        "abc_p": abc,
    }


_RUN_KWARGS = {}
_LAST_RESULT = None


def kernel(prior_mean, prior_logvar, post_mean, post_logvar, eps):
    global _CACHED_NC, _LAST_RESULT
    prior_mean = np.asarray(prior_mean, dtype=np.float32)
    prior_logvar = np.asarray(prior_logvar, dtype=np.float32)
    post_mean = np.asarray(post_mean, dtype=np.float32)
    post_logvar = np.asarray(post_logvar, dtype=np.float32)
    eps = np.asarray(eps, dtype=np.float32)

    if _CACHED_NC is None:
        _CACHED_NC = _build_nc()
    nc = _CACHED_NC

    coeffs = _host_coeffs(post_mean, post_logvar)
    in_maps = [
        _prep_core_inputs(prior_mean, prior_logvar, post_mean, post_logvar,
                          eps, c, coeffs)
        for c in range(NCORES)
    ]
    res = run_bass_kernel_spmd(nc, in_maps, core_ids=list(range(NCORES)),
                               **_RUN_KWARGS)
    _LAST_RESULT = res

    tot = 0.0
    for c in range(NCORES):
        o = np.asarray(res.results[c]["out"], dtype=np.float64)
        # log-sum column is 32x replicated across each partition block
        tot += o[:, 0].sum() / 32.0 - o[:, 1].sum()
    kl = (tot - B * D * S * np.log(B)) / (B * S)
    return np.float32(kl)


# revision 4
# speedup vs baseline: 4.2537x; 2.3320x over previous
"""LPO loss kernel v2 for 8 TRN2 NeuronCores.

Math (B=256, D=64, S=32):
  zs[j,d,s] = post_mean[j,d] + eps[j,d,s]*exp(0.5*post_logvar[j,d])
  logp_post[i,j,d,s] = A0[i,d] + A1[i,d]*z + A2[i,d]*z^2
  lagg[j,d,s] = log(sum_i exp(logp_post)) - log(B)
  kl = sum_{j,d,s}(lagg - logp_prior) / (B*S)

Sharding: j split 8 ways; i-reduction local; host combines scalars.

v2 dataflow (per core), swapped-operand layout:
  TensorE mains: out[i(128 of 256), js 512] = amat[6, i128]^T @ zmat[6, js512]
                 (bf16 hi/lo of A1*z + A2*z^2; A0 folded into exp bias)
  exp:   ScalarE activation(Exp, bias=A0col) on ~62.5% of (d, ihalf) tiles
         DVE Schraudolph on the rest, in bf16 bit-space:
         u16((ps + A0 + B/A) * 2^7/ln2) bitcast to bf16 (fp32->uint
         converts saturate negatives to 0 on TRN2 = exact underflow)
  i-fold: TensorE replicated-ones matmul [32,512] blocks, 2 accumulating
          matmuls (i-halves) per js chunk; 4 pairs fill a PSUM bank, then
          one Ln+accum per bank (host divides the 32x replication out)
  A-coefficients (A0/A1/A2 hi-lo splits) are precomputed on host ([B,D]
  sized, negligible); z/z2 and the prior logpdf run on device.
"""

import sys

sys.path.insert(0, "/opt/trn_rl_repo")

import numpy as np
import ml_dtypes

import concourse.bass as bass
import concourse.bacc as bacc
import concourse.mybir as mybir
from concourse import tile
from concourse.bass_utils import run_bass_kernel_spmd

# Route both Exp and Ln to the one table set that holds them both
# ("natural_log_exp_and_others") so the kernel pays a single
# ACT_TABLE_LOAD instead of swapping sets between exp and ln calls.
# Order of the dict is preserved, so act_func_set_ids stay valid.
_orig_get_tables = bacc.get_activation_tables


def _patched_get_tables(arch):
    tabs = dict(_orig_get_tables(arch))
    exp_ln = {mybir.ActivationFunctionType.Exp, mybir.ActivationFunctionType.Ln}
    combined = None
    for name, fns in tabs.items():
        if exp_ln <= fns:
            combined = name
            break
    if combined is not None:
        for name in tabs:
            if name != combined:
                tabs[name] = tabs[name] - exp_ln
    return tabs


bacc.get_activation_tables = _patched_get_tables

B, D, S = 256, 64, 32
NCORES = 8
BJ = B // NCORES          # 32 j's per core
JS = BJ * S               # 1024 js columns per core
J2 = JS // 2              # [D, JS] reshaped to [128, J2] (same bytes)
DQ = D // 4               # 16 d's per zmat quarter
L2P = float(np.log(2.0 * np.pi))
LN2 = float(np.log(2.0))
C0 = -0.5 * L2P
F32 = mybir.dt.float32
F32R = mybir.dt.float32r
BF16 = mybir.dt.bfloat16
U16 = mybir.dt.uint16
AF = mybir.ActivationFunctionType
ALU = mybir.AluOpType
bf = ml_dtypes.bfloat16

# Schraudolph exp in bf16 bit-space: exp(x) ~= bitcast_bf16(u16(x*A + B));
# the fp32->uint convert saturates negatives to 0 on TRN2 (verified on hw),
# which handles exp underflow exactly.
A_SCH = float(2.0**7 / np.log(2.0))
C_SCH = 7.37              # mean-error-nulling constant (tuned numerically)
B_SCH = float(127 * 2**7) - C_SCH


def _dve_tile(d, ih):
    # which (d, i-half) exp tiles go to DVE-Schraudolph: 37.5% of tiles
    return ih == 1 and (d % 4 != 0)


_CACHED_NC = None


def _build_nc(reps=1):
    """reps>1 repeats the whole compute (idempotent) inside one NEFF so
    wall-clock deltas between rep counts isolate true HW exec time."""
    nc = bacc.Bacc(None)

    for val in (0.5, LN2):
        t = nc.alloc_sbuf_tensor(f"const-user-{val}", [128, 1], F32)
        nc.gpsimd.memset(t.ap(), val)
        nc.const_aps.aps[(F32, val)] = t.ap()
    nc.all_engine_barrier()

    eps64 = nc.declare_dram_parameter("eps64", [128, J2], F32, isOutput=False)
    mj64 = nc.declare_dram_parameter("mj64", [128, J2], F32, isOutput=False)
    lvj64 = nc.declare_dram_parameter("lvj64", [128, J2], F32, isOutput=False)
    pm64 = nc.declare_dram_parameter("pm64", [128, J2], F32, isOutput=False)
    plv64 = nc.declare_dram_parameter("plv64", [128, J2], F32, isOutput=False)
    amat_p = nc.declare_dram_parameter("amat_p", [6, D * B], BF16, isOutput=False)
    a0c_p = nc.declare_dram_parameter("a0c_p", [128, D * 2], F32, isOutput=False)
    abc_p = nc.declare_dram_parameter("abc_p", [128, D * 2], F32, isOutput=False)
    out = nc.declare_dram_parameter("out", [128, 2], F32, isOutput=True)

    with tile.TileContext(nc) as tc:
        with (
            tc.tile_pool(name="persist", bufs=1) as pp,
            tc.tile_pool(name="prep", bufs=1) as prep,
            tc.tile_pool(name="dram", bufs=1, space="DRAM") as dram,
            tc.tile_pool(name="psum", bufs=3, space="PSUM") as psp,
            tc.tile_pool(name="snk", bufs=2, space="PSUM") as snk,
            tc.tile_pool(name="expp", bufs=6) as expp,
            tc.tile_pool(name="sup", bufs=4) as sup,
            tc.tile_pool(name="lasc", bufs=2) as lasc,
            tc.tile_pool(name="zqp", bufs=2) as zqp,
        ):
            # ---------------- persistent tensors ----------------
            amat = pp.tile([6, D * B], BF16, tag="amat")        # 32KB rows 0-5
            a0c = pp.tile([128, D * 2], F32, tag="a0c")         # A0[i,(d,ih)]
            abc = pp.tile([128, D * 2], F32, tag="abc")         # A0 + B/A
            ones_b = pp.tile([128, 32], BF16, tag="ones_b")
            acc_cols = pp.tile([128, 32], F32, tag="acc_cols")
            outt = pp.tile([128, 2], F32, tag="outt")

            nc.gpsimd.memset(ones_b[:, :], 1.0)

            # DRAM bounce buffers for the partition->free zmat row gathers
            # (the "(o p) j -> o (p j)" rearrange needs linear memory)
            zh_d = dram.tile([128, J2], BF16, tag="zh_d")
            zl_d = dram.tile([128, J2], BF16, tag="zl_d")
            z2h_d = dram.tile([128, J2], BF16, tag="z2h_d")
            z2l_d = dram.tile([128, J2], BF16, tag="z2l_d")

            # ---------------- prep tiles ----------------
            t_eps = prep.tile([128, J2], F32, tag="t_eps")
            t_mj = prep.tile([128, J2], F32, tag="t_mj")
            t_lvj = prep.tile([128, J2], F32, tag="t_lvj")
            t_pm = prep.tile([128, J2], F32, tag="t_pm")
            t_plv = prep.tile([128, J2], F32, tag="t_plv")
            t_std = prep.tile([128, J2], F32, tag="t_std")
            t_z = prep.tile([128, J2], F32, tag="t_z")
            t_z2 = prep.tile([128, J2], F32, tag="t_z2")
            t_r = prep.tile([128, J2], F32, tag="t_r")
            t_lp = prep.tile([128, J2], F32, tag="t_lp")
            t_wp = prep.tile([128, J2], F32, tag="t_wp")
            zh = prep.tile([128, J2], BF16, tag="zh")
            zl = prep.tile([128, J2], BF16, tag="zl")
            z2h = prep.tile([128, J2], BF16, tag="z2h")
            z2l = prep.tile([128, J2], BF16, tag="z2l")
            lpsum = prep.tile([128, 1], F32, tag="lpsum")

            # loads needed before the main loop
            nc.sync.dma_start(t_eps[:, :], eps64[:, :])
            nc.sync.dma_start(t_mj[:, :], mj64[:, :])
            nc.sync.dma_start(t_lvj[:, :], lvj64[:, :])
            nc.sync.dma_start(amat[:, :], amat_p[:, :])
            nc.sync.dma_start(a0c[:, :], a0c_p[:, :])
            nc.sync.dma_start(abc[:, :], abc_p[:, :])

            # ---------------- z path ----------------
            # z = mj + eps*exp(0.5*lvj); z2 = z*z
            nc.scalar.activation(t_std[:, :], t_lvj[:, :], AF.Exp, scale=0.5)
            nc.vector.tensor_mul(t_z[:, :], t_eps[:, :], t_std[:, :])
            nc.vector.tensor_add(t_z[:, :], t_z[:, :], t_mj[:, :])
            nc.vector.tensor_mul(t_z2[:, :], t_z[:, :], t_z[:, :])
            # hi/lo splits
            nc.vector.tensor_copy(zh[:, :], t_z[:, :])
            nc.vector.tensor_sub(t_r[:, :], t_z[:, :], zh[:, :])
            nc.vector.tensor_copy(zl[:, :], t_r[:, :])
            nc.vector.tensor_copy(z2h[:, :], t_z2[:, :])
            nc.vector.tensor_sub(t_r[:, :], t_z2[:, :], z2h[:, :])
            nc.vector.tensor_copy(z2l[:, :], t_r[:, :])
            nc.sync.dma_start(zh_d[:, :], zh[:, :])
            nc.sync.dma_start(zl_d[:, :], zl[:, :])
            nc.sync.dma_start(z2h_d[:, :], z2h[:, :])
            nc.sync.dma_start(z2l_d[:, :], z2l[:, :])

            # ---------------- main loop ----------------
            def build_zmat_quarter(qi):
                # rows 0,1 = zh ; 2 = zl ; 3,4 = z2h ; 5 = z2l
                # (quarter qi = DRAM partitions [qi*32, qi*32+32))
                zq = zqp.tile([6, DQ * JS], BF16, tag="zq", name="zq")
                psl = slice(qi * 32, (qi + 1) * 32)
                for row, src in ((0, zh_d), (1, zh_d), (2, zl_d),
                                 (3, z2h_d), (4, z2h_d), (5, z2l_d)):
                    nc.sync.dma_start(
                        zq[row:row + 1, :],
                        src[psl, :].rearrange("(o p) j -> o (p j)", o=1))
                return zq

            sums_bank = [None]  # current [128, 512] PSUM sums bank

            def issue_ones(d, rhs_slc):
                # i-fold: 2 accumulating ones-matmuls per js chunk;
                # pair q=(d*2+jc) lands in 32-part block q%4 of the bank
                for jc in range(2):
                    q = d * 2 + jc
                    blk = q % 4
                    if blk == 0:
                        sums_bank[0] = snk.tile([128, 512], F32,
                                                tag="snk", name="snk")
                    for ih in range(2):
                        nc.tensor.matmul(
                            sums_bank[0][blk * 32:(blk + 1) * 32, :],
                            ones_b[:, :],
                            rhs_slc[ih][:, jc * 512:(jc + 1) * 512],
                            start=(ih == 0), stop=(ih == 1),
                            tile_position=(0, blk * 32))
                if d % 2 == 1:
                    # bank (d-1)//2 is complete: Ln + accum -> acc column
                    b = (d - 1) // 2
                    lsc = lasc.tile([128, 512], BF16, tag="lsc", name="lsc")
                    nc.scalar.activation(lsc[:, :], sums_bank[0][:, :], AF.Ln,
                                         accum_out=acc_cols[:, b:b + 1])
                    sums_bank[0] = None

            pend = None       # (d, rhs_slc) whose ones-fold is deferred
            zq = None
            for d in range(D):
                dq = d % DQ
                if dq == 0:
                    zq = build_zmat_quarter(d // DQ)
                # per (d, i-half): matmul both js chunks, then exp
                rhs_slc = [None, None]   # rhs APs for the ones-fold
                for ih in range(2):
                    ps = psp.tile([128, 2 * 512], F32, tag="ps")
                    for jc in range(2):
                        nc.tensor.matmul(
                            ps[:, jc * 512:(jc + 1) * 512],
                            amat[0:6, d * B + ih * 128:
                                 d * B + ih * 128 + 128],
                            zq[0:6, dq * JS + jc * 512:
                               dq * JS + (jc + 1) * 512],
                            start=True, stop=True)
                    col = d * 2 + ih
                    if _dve_tile(d, ih):
                        su = sup.tile([128, 2 * 512], U16, tag="su")
                        nc.vector.scalar_tensor_tensor(
                            su[:, :], ps[:, :], abc[:, col:col + 1],
                            acst[:, 0:1024],
                            op0=ALU.add, op1=ALU.mult)
                        rhs_slc[ih] = su[:, :].bitcast(BF16)
                    else:
                        ex = expp.tile([128, 2 * 512], BF16, tag="ex")
                        nc.scalar.activation(ex[:, :], ps[:, :], AF.Exp,
                                             bias=a0c[:, col:col + 1])
                        rhs_slc[ih] = ex[:, :]
                # software pipelining: fold the PREVIOUS d now, so the PE
                # never waits on this d's exp in program order
                if pend is not None:
                    issue_ones(*pend)
                pend = (d, rhs_slc)
            issue_ones(*pend)

            # ---- prior logpdf + partial sum (overlaps the loop tail)
            # lp = C0 -0.5*plv -(z-pm)^2/(2e^plv+1e-4)
            nc.sync.dma_start(t_pm[:, :], pm64[:, :])
            nc.sync.dma_start(t_plv[:, :], plv64[:, :])
            nc.scalar.activation(t_wp[:, :], t_plv[:, :], AF.Exp, bias=LN2)
            nc.vector.tensor_scalar_add(t_wp[:, :], t_wp[:, :], 1e-4)
            nc.vector.reciprocal(t_wp[:, :], t_wp[:, :])
            nc.vector.tensor_sub(t_lp[:, :], t_z[:, :], t_pm[:, :])
            nc.vector.tensor_mul(t_lp[:, :], t_lp[:, :], t_lp[:, :])
            nc.vector.tensor_mul(t_lp[:, :], t_lp[:, :], t_wp[:, :])
            nc.vector.tensor_scalar(t_plv[:, :], t_plv[:, :], -0.5, C0,
                                    op0=ALU.mult, op1=ALU.add)
            nc.vector.tensor_sub(t_lp[:, :], t_plv[:, :], t_lp[:, :])
            nc.vector.reduce_sum(lpsum[:, :], t_lp[:, :],
                                 axis=mybir.AxisListType.X)

            # ---------------- tail: combine partials ----------------
            nc.vector.reduce_sum(outt[:, 0:1], acc_cols[:, :],
                                 axis=mybir.AxisListType.X)
            nc.vector.tensor_copy(outt[:, 1:2], lpsum[:, :])
            nc.sync.dma_start(out[:, :], outt[:, :])

    nc.compile()
    return nc


def _host_coeffs(post_mean, post_logvar):
    """amat rows (A1h, A1l, A1h, A2h, A2l, A2h) in [6, (d, i)] bf16 layout,
    plus A0 bias columns a0c[p, (d, ih)] and abc = a0c + B/A."""
    m = post_mean.astype(np.float64)        # [B, D]
    lv = post_logvar.astype(np.float64)
    w = 1.0 / (2.0 * np.exp(lv) + 1e-4)
    A1 = 2.0 * m * w
    A2 = -w
    A0 = C0 - 0.5 * lv - m * m * w

    def hilo(x):  # [B, D] -> bf16 hi + lo
        hi = x.astype(np.float32).astype(bf)
        lo = (x - hi.astype(np.float64)).astype(np.float32).astype(bf)
        return hi, lo

    a1h, a1l = hilo(A1)
    a2h, a2l = hilo(A2)
    # amat row layout: free index = d*B + i  (i = ih*128 + p)
    rows = [a1h, a1l, a1h, a2h, a2l, a2h]
    amat = np.stack([np.ascontiguousarray(r.T).reshape(-1) for r in rows])
    # a0c[p, d*2 + ih] = A0[ih*128 + p, d]
    a0c = np.empty((128, D * 2), np.float32)
    for ih in range(2):
        a0c[:, ih::2] = A0[ih * 128:(ih + 1) * 128, :].astype(np.float32)
    abc = (a0c.astype(np.float64) + B_SCH / A_SCH).astype(np.float32)
    return amat.astype(bf), a0c, abc


def _prep_core_inputs(prior_mean, prior_logvar, post_mean, post_logvar, eps,
                      c, coeffs):
    jsl = slice(c * BJ, (c + 1) * BJ)

    def b64(x):  # [BJ, D] -> [D, JS] broadcast over s -> [128, J2]
        return np.ascontiguousarray(
            np.broadcast_to(x.T[:, :, None], (D, BJ, S)).reshape(128, J2),
            dtype=np.float32)

    amat, a0c, abc = coeffs
    eps64 = np.ascontiguousarray(
        eps[jsl].transpose(1, 0, 2).reshape(128, J2), dtype=np.float32)
    return {
        "eps64": eps64,
        "mj64": b64(post_mean[jsl]),
        "lvj64": b64(post_logvar[jsl]),
        "pm64": b64(prior_mean[jsl]),
        "plv64": b64(prior_logvar[jsl]),
        "amat_p": amat,
        "a0c_p": a0c,
        "abc_p": abc,
    }


_RUN_KWARGS = {}
_LAST_RESULT = None


def kernel(prior_mean, prior_logvar, post_mean, post_logvar, eps):
    global _CACHED_NC, _LAST_RESULT
    prior_mean = np.asarray(prior_mean, dtype=np.float32)
    prior_logvar = np.asarray(prior_logvar, dtype=np.float32)
    post_mean = np.asarray(post_mean, dtype=np.float32)
    post_logvar = np.asarray(post_logvar, dtype=np.float32)
    eps = np.asarray(eps, dtype=np.float32)

    if _CACHED_NC is None:
        _CACHED_NC = _build_nc()
    nc = _CACHED_NC

    coeffs = _host_coeffs(post_mean, post_logvar)
    in_maps = [
        _prep_core_inputs(prior_mean, prior_logvar, post_mean, post_logvar,
                          eps, c, coeffs)
        for c in range(NCORES)
    ]
    res = run_bass_kernel_spmd(nc, in_maps, core_ids=list(range(NCORES)),
                               **_RUN_KWARGS)
    _LAST_RESULT = res

    tot = 0.0
    for c in range(NCORES):
        o = np.asarray(res.results[c]["out"], dtype=np.float64)
        # log-sum column is 32x replicated across each partition block
        tot += o[:, 0].sum() / 32.0 - o[:, 1].sum()
    kl = (tot - B * D * S * np.log(B)) / (B * S)
    return np.float32(kl)


# revision 5
# speedup vs baseline: 80.4746x; 18.9188x over previous
"""LPO loss kernel v2 for 8 TRN2 NeuronCores.

Math (B=256, D=64, S=32):
  zs[j,d,s] = post_mean[j,d] + eps[j,d,s]*exp(0.5*post_logvar[j,d])
  logp_post[i,j,d,s] = A0[i,d] + A1[i,d]*z + A2[i,d]*z^2
  lagg[j,d,s] = log(sum_i exp(logp_post)) - log(B)
  kl = sum_{j,d,s}(lagg - logp_prior) / (B*S)

Sharding: j split 8 ways; i-reduction local; host combines scalars.

v2 dataflow (per core), swapped-operand layout:
  TensorE mains: out[i(128 of 256), js 512] = amat[6, i128]^T @ zmat[6, js512]
                 (bf16 hi/lo of A1*z + A2*z^2; A0 folded into exp bias)
  exp:   ScalarE activation(Exp, bias=A0col) on ~62.5% of (d, ihalf) tiles
         DVE Schraudolph on the rest, in bf16 bit-space:
         u16((ps + A0 + B/A) * 2^7/ln2) bitcast to bf16 (fp32->uint
         converts saturate negatives to 0 on TRN2 = exact underflow)
  i-fold: TensorE replicated-ones matmul [32,512] blocks, 2 accumulating
          matmuls (i-halves) per js chunk; 4 pairs fill a PSUM bank, then
          one Ln+accum per bank (host divides the 32x replication out)
  A-coefficients (A0/A1/A2 hi-lo splits) are precomputed on host ([B,D]
  sized, negligible); z/z2 and the prior logpdf run on device.
"""

import sys

sys.path.insert(0, "/opt/trn_rl_repo")

import numpy as np
import ml_dtypes

import concourse.bass as bass
import concourse.bacc as bacc
import concourse.mybir as mybir
from concourse import tile
from concourse.bass_utils import run_bass_kernel_spmd

# Route both Exp and Ln to the one table set that holds them both
# ("natural_log_exp_and_others") so the kernel pays a single
# ACT_TABLE_LOAD instead of swapping sets between exp and ln calls.
# Order of the dict is preserved, so act_func_set_ids stay valid.
_orig_get_tables = bacc.get_activation_tables


def _patched_get_tables(arch):
    tabs = dict(_orig_get_tables(arch))
    exp_ln = {mybir.ActivationFunctionType.Exp, mybir.ActivationFunctionType.Ln}
    combined = None
    for name, fns in tabs.items():
        if exp_ln <= fns:
            combined = name
            break
    if combined is not None:
        for name in tabs:
            if name != combined:
                tabs[name] = tabs[name] - exp_ln
    return tabs


bacc.get_activation_tables = _patched_get_tables

B, D, S = 256, 64, 32
NCORES = 8
BJ = B // NCORES          # 32 j's per core
JS = BJ * S               # 1024 js columns per core
J2 = JS // 2              # [D, JS] reshaped to [128, J2] (same bytes)
DQ = D // 4               # 16 d's per zmat quarter
L2P = float(np.log(2.0 * np.pi))
LN2 = float(np.log(2.0))
C0 = -0.5 * L2P
F32 = mybir.dt.float32
F32R = mybir.dt.float32r
BF16 = mybir.dt.bfloat16
U16 = mybir.dt.uint16
AF = mybir.ActivationFunctionType
ALU = mybir.AluOpType
bf = ml_dtypes.bfloat16

# Schraudolph exp in bf16 bit-space: exp(x) ~= bitcast_bf16(u16(x*A + B));
# the fp32->uint convert saturates negatives to 0 on TRN2 (verified on hw),
# which handles exp underflow exactly.
A_SCH = float(2.0**7 / np.log(2.0))
C_SCH = 7.37              # mean-error-nulling constant (tuned numerically)
B_SCH = float(127 * 2**7) - C_SCH


def _dve_tile(d, ih):
    # which (d, i-half) exp tiles go to DVE-Schraudolph: 37.5% of tiles
    return ih == 1 and (d % 4 != 0)


_CACHED_NC = None


def _build_nc(reps=1):
    """reps>1 repeats the whole compute (idempotent) inside one NEFF so
    wall-clock deltas between rep counts isolate true HW exec time."""
    nc = bacc.Bacc(None)

    for val in (0.5, LN2):
        t = nc.alloc_sbuf_tensor(f"const-user-{val}", [128, 1], F32)
        nc.gpsimd.memset(t.ap(), val)
        nc.const_aps.aps[(F32, val)] = t.ap()
    nc.all_engine_barrier()

    eps64 = nc.declare_dram_parameter("eps64", [128, J2], F32, isOutput=False)
    mj64 = nc.declare_dram_parameter("mj64", [128, J2], F32, isOutput=False)
    lvj64 = nc.declare_dram_parameter("lvj64", [128, J2], F32, isOutput=False)
    pm64 = nc.declare_dram_parameter("pm64", [128, J2], F32, isOutput=False)
    plv64 = nc.declare_dram_parameter("plv64", [128, J2], F32, isOutput=False)
    amat_p = nc.declare_dram_parameter("amat_p", [6, D * B], BF16, isOutput=False)
    a0c_p = nc.declare_dram_parameter("a0c_p", [128, D * 2], F32, isOutput=False)
    abc_p = nc.declare_dram_parameter("abc_p", [128, D * 2], F32, isOutput=False)
    out = nc.declare_dram_parameter("out", [128, 2], F32, isOutput=True)

    with tile.TileContext(nc) as tc:
        with (
            tc.tile_pool(name="persist", bufs=1) as pp,
            tc.tile_pool(name="prep", bufs=1) as prep,
            tc.tile_pool(name="dram", bufs=1, space="DRAM") as dram,
            tc.tile_pool(name="psum", bufs=3, space="PSUM") as psp,
            tc.tile_pool(name="snk", bufs=2, space="PSUM") as snk,
            tc.tile_pool(name="expp", bufs=6) as expp,
            tc.tile_pool(name="sup", bufs=4) as sup,
            tc.tile_pool(name="lasc", bufs=2) as lasc,
            tc.tile_pool(name="exsp", bufs=4) as exsp,
            tc.tile_pool(name="zqp", bufs=2) as zqp,
        ):
            # ---------------- persistent tensors ----------------
            amat = pp.tile([6, D * B], BF16, tag="amat")        # 32KB rows 0-5
            a0c = pp.tile([128, D * 2], F32, tag="a0c")         # A0[i,(d,ih)]
            abc = pp.tile([128, D * 2], F32, tag="abc")         # A0 + B/A
            ones_b = pp.tile([128, 32], BF16, tag="ones_b")
            acc_cols = pp.tile([128, 32], F32, tag="acc_cols")
            outt = pp.tile([128, 2], F32, tag="outt")

            nc.gpsimd.memset(ones_b[:, :], 1.0)

            # DRAM bounce buffers for the partition->free zmat row gathers
            # (the "(o p) j -> o (p j)" rearrange needs linear memory)
            zh_d = dram.tile([128, J2], BF16, tag="zh_d")
            zl_d = dram.tile([128, J2], BF16, tag="zl_d")
            z2h_d = dram.tile([128, J2], BF16, tag="z2h_d")
            z2l_d = dram.tile([128, J2], BF16, tag="z2l_d")

            # ---------------- prep tiles ----------------
            t_eps = prep.tile([128, J2], F32, tag="t_eps")
            t_mj = prep.tile([128, J2], F32, tag="t_mj")
            t_lvj = prep.tile([128, J2], F32, tag="t_lvj")
            t_pm = prep.tile([128, J2], F32, tag="t_pm")
            t_plv = prep.tile([128, J2], F32, tag="t_plv")
            t_std = prep.tile([128, J2], F32, tag="t_std")
            t_z = prep.tile([128, J2], F32, tag="t_z")
            t_z2 = prep.tile([128, J2], F32, tag="t_z2")
            t_r = prep.tile([128, J2], F32, tag="t_r")
            t_lp = prep.tile([128, J2], F32, tag="t_lp")
            t_wp = prep.tile([128, J2], F32, tag="t_wp")
            zh = prep.tile([128, J2], BF16, tag="zh")
            zl = prep.tile([128, J2], BF16, tag="zl")
            z2h = prep.tile([128, J2], BF16, tag="z2h")
            z2l = prep.tile([128, J2], BF16, tag="z2l")
            lpsum = prep.tile([128, 1], F32, tag="lpsum")

            # loads needed before the main loop
            nc.sync.dma_start(t_eps[:, :], eps64[:, :])
            nc.sync.dma_start(t_mj[:, :], mj64[:, :])
            nc.sync.dma_start(t_lvj[:, :], lvj64[:, :])
            nc.sync.dma_start(amat[:, :], amat_p[:, :])
            nc.sync.dma_start(a0c[:, :], a0c_p[:, :])
            nc.sync.dma_start(abc[:, :], abc_p[:, :])

            # ---------------- z path ----------------
            # z = mj + eps*exp(0.5*lvj); z2 = z*z
            nc.scalar.activation(t_std[:, :], t_lvj[:, :], AF.Exp, scale=0.5)
            nc.vector.tensor_mul(t_z[:, :], t_eps[:, :], t_std[:, :])
            nc.vector.tensor_add(t_z[:, :], t_z[:, :], t_mj[:, :])
            nc.vector.tensor_mul(t_z2[:, :], t_z[:, :], t_z[:, :])
            # hi/lo splits
            nc.vector.tensor_copy(zh[:, :], t_z[:, :])
            nc.vector.tensor_sub(t_r[:, :], t_z[:, :], zh[:, :])
            nc.vector.tensor_copy(zl[:, :], t_r[:, :])
            nc.vector.tensor_copy(z2h[:, :], t_z2[:, :])
            nc.vector.tensor_sub(t_r[:, :], t_z2[:, :], z2h[:, :])
            nc.vector.tensor_copy(z2l[:, :], t_r[:, :])
            nc.sync.dma_start(zh_d[:, :], zh[:, :])
            nc.sync.dma_start(zl_d[:, :], zl[:, :])
            nc.sync.dma_start(z2h_d[:, :], z2h[:, :])
            nc.sync.dma_start(z2l_d[:, :], z2l[:, :])

            # ---------------- main loop ----------------
            def build_zmat_quarter(qi):
                # rows 0,1 = zh ; 2 = zl ; 3,4 = z2h ; 5 = z2l
                # (quarter qi = DRAM partitions [qi*32, qi*32+32))
                zq = zqp.tile([6, DQ * JS], BF16, tag="zq", name="zq")
                psl = slice(qi * 32, (qi + 1) * 32)
                for row, src in ((0, zh_d), (1, zh_d), (2, zl_d),
                                 (3, z2h_d), (4, z2h_d), (5, z2l_d)):
                    nc.sync.dma_start(
                        zq[row:row + 1, :],
                        src[psl, :].rearrange("(o p) j -> o (p j)", o=1))
                return zq

            sums_bank = [None]  # current [128, 512] PSUM sums bank

            def issue_ones(d, rhs_slc):
                # i-fold: 2 accumulating ones-matmuls per js chunk;
                # pair q=(d*2+jc) lands in 32-part block q%4 of the bank
                for jc in range(2):
                    q = d * 2 + jc
                    blk = q % 4
                    if blk == 0:
                        sums_bank[0] = snk.tile([128, 512], F32,
                                                tag="snk", name="snk")
                    for ih in range(2):
                        nc.tensor.matmul(
                            sums_bank[0][blk * 32:(blk + 1) * 32, :],
                            ones_b[:, :],
                            rhs_slc[ih][:, jc * 512:(jc + 1) * 512],
                            start=(ih == 0), stop=(ih == 1),
                            tile_position=(0, blk * 32))
                if d % 2 == 1:
                    # bank (d-1)//2 is complete: Ln + accum -> acc column
                    b = (d - 1) // 2
                    lsc = lasc.tile([128, 512], BF16, tag="lsc", name="lsc")
                    nc.scalar.activation(lsc[:, :], sums_bank[0][:, :], AF.Ln,
                                         accum_out=acc_cols[:, b:b + 1])
                    sums_bank[0] = None

            pend = None       # (d, rhs_slc) whose ones-fold is deferred
            zq = None
            for d in range(D):
                dq = d % DQ
                if dq == 0:
                    zq = build_zmat_quarter(d // DQ)
                # per (d, i-half): matmul both js chunks, then exp
                rhs_slc = [None, None]   # rhs APs for the ones-fold
                for ih in range(2):
                    ps = psp.tile([128, 2 * 512], F32, tag="ps")
                    for jc in range(2):
                        nc.tensor.matmul(
                            ps[:, jc * 512:(jc + 1) * 512],
                            amat[0:6, d * B + ih * 128:
                                 d * B + ih * 128 + 128],
                            zq[0:6, dq * JS + jc * 512:
                               dq * JS + (jc + 1) * 512],
                            start=True, stop=True)
                    col = d * 2 + ih
                    if _dve_tile(d, ih):
                        su = sup.tile([128, 2 * 512], U16, tag="su")
                        nc.vector.scalar_tensor_tensor(
                            su[:, :], ps[:, :], abc[:, col:col + 1],
                            acst[:, 0:1024],
                            op0=ALU.add, op1=ALU.mult)
                        rhs_slc[ih] = su[:, :].bitcast(BF16)
                    else:
                        ex = expp.tile([128, 2 * 512], BF16, tag="ex")
                        nc.scalar.activation(ex[:, :], ps[:, :], AF.Exp,
                                             bias=a0c[:, col:col + 1])
                        rhs_slc[ih] = ex[:, :]
                # software pipelining: fold the PREVIOUS d now, so the PE
                # never waits on this d's exp in program order
                if pend is not None:
                    issue_ones(*pend)
                pend = (d, rhs_slc)
            issue_ones(*pend)

            # ---- prior logpdf + partial sum (overlaps the loop tail)
            # lp = C0 -0.5*plv -(z-pm)^2/(2e^plv+1e-4)
            nc.sync.dma_start(t_pm[:, :], pm64[:, :])
            nc.sync.dma_start(t_plv[:, :], plv64[:, :])
            nc.scalar.activation(t_wp[:, :], t_plv[:, :], AF.Exp, bias=LN2)
            nc.vector.tensor_scalar_add(t_wp[:, :], t_wp[:, :], 1e-4)
            nc.vector.reciprocal(t_wp[:, :], t_wp[:, :])
            nc.vector.tensor_sub(t_lp[:, :], t_z[:, :], t_pm[:, :])
            nc.vector.tensor_mul(t_lp[:, :], t_lp[:, :], t_lp[:, :])
            nc.vector.tensor_mul(t_lp[:, :], t_lp[:, :], t_wp[:, :])
            nc.vector.tensor_scalar(t_plv[:, :], t_plv[:, :], -0.5, C0,
                                    op0=ALU.mult, op1=ALU.add)
            nc.vector.tensor_sub(t_lp[:, :], t_plv[:, :], t_lp[:, :])
            nc.vector.reduce_sum(lpsum[:, :], t_lp[:, :],
                                 axis=mybir.AxisListType.X)

            # ---------------- tail: combine partials ----------------
            nc.vector.reduce_sum(outt[:, 0:1], acc_cols[:, :],
                                 axis=mybir.AxisListType.X)
            nc.vector.tensor_copy(outt[:, 1:2], lpsum[:, :])
            nc.sync.dma_start(out[:, :], outt[:, :])

    nc.compile()
    return nc


def _host_coeffs(post_mean, post_logvar):
    """amat rows (A1h, A1l, A1h, A2h, A2l, A2h) in [6, (d, i)] bf16 layout,
    plus A0 bias columns a0c[p, (d, ih)] and abc = a0c + B/A."""
    m = post_mean.astype(np.float64)        # [B, D]
    lv = post_logvar.astype(np.float64)
    w = 1.0 / (2.0 * np.exp(lv) + 1e-4)
    A1 = 2.0 * m * w
    A2 = -w
    A0 = C0 - 0.5 * lv - m * m * w

    def hilo(x):  # [B, D] -> bf16 hi + lo
        hi = x.astype(np.float32).astype(bf)
        lo = (x - hi.astype(np.float64)).astype(np.float32).astype(bf)
        return hi, lo

    a1h, a1l = hilo(A1)
    a2h, a2l = hilo(A2)
    # amat row layout: free index = d*B + i  (i = ih*128 + p)
    rows = [a1h, a1l, a1h, a2h, a2l, a2h]
    amat = np.stack([np.ascontiguousarray(r.T).reshape(-1) for r in rows])
    # a0c[p, d*2 + ih] = A0[ih*128 + p, d]
    a0c = np.empty((128, D * 2), np.float32)
    for ih in range(2):
        a0c[:, ih::2] = A0[ih * 128:(ih + 1) * 128, :].astype(np.float32)
    abc = (a0c.astype(np.float64) + B_SCH / A_SCH).astype(np.float32)
    return amat.astype(bf), a0c, abc


def _prep_core_inputs(prior_mean, prior_logvar, post_mean, post_logvar, eps,
                      c, coeffs):
    jsl = slice(c * BJ, (c + 1) * BJ)

    def b64(x):  # [BJ, D] -> [D, JS] broadcast over s -> [128, J2]
        return np.ascontiguousarray(
            np.broadcast_to(x.T[:, :, None], (D, BJ, S)).reshape(128, J2),
            dtype=np.float32)

    amat, a0c, abc = coeffs
    eps64 = np.ascontiguousarray(
        eps[jsl].transpose(1, 0, 2).reshape(128, J2), dtype=np.float32)
    return {
        "eps64": eps64,
        "mj64": b64(post_mean[jsl]),
        "lvj64": b64(post_logvar[jsl]),
        "pm64": b64(prior_mean[jsl]),
        "plv64": b64(prior_logvar[jsl]),
        "amat_p": amat,
        "a0c_p": a0c,
        "abc_p": abc,
    }


_RUN_KWARGS = {}
_LAST_RESULT = None


def kernel(prior_mean, prior_logvar, post_mean, post_logvar, eps):
    global _CACHED_NC, _LAST_RESULT
    prior_mean = np.asarray(prior_mean, dtype=np.float32)
    prior_logvar = np.asarray(prior_logvar, dtype=np.float32)
    post_mean = np.asarray(post_mean, dtype=np.float32)
    post_logvar = np.asarray(post_logvar, dtype=np.float32)
    eps = np.asarray(eps, dtype=np.float32)

    if _CACHED_NC is None:
        _CACHED_NC = _build_nc()
    nc = _CACHED_NC

    coeffs = _host_coeffs(post_mean, post_logvar)
    in_maps = [
        _prep_core_inputs(prior_mean, prior_logvar, post_mean, post_logvar,
                          eps, c, coeffs)
        for c in range(NCORES)
    ]
    res = run_bass_kernel_spmd(nc, in_maps, core_ids=list(range(NCORES)),
                               **_RUN_KWARGS)
    _LAST_RESULT = res

    tot = 0.0
    for c in range(NCORES):
        o = np.asarray(res.results[c]["out"], dtype=np.float64)
        # log-sum column is 32x replicated across each partition block
        tot += o[:, 0].sum() / 32.0 - o[:, 1].sum()
    kl = (tot - B * D * S * np.log(B)) / (B * S)
    return np.float32(kl)
